# revision 23
# baseline (speedup 1.0000x reference)
"""CrossViewTransformer Bass kernel for 8 trn2 NeuronCores (v9).

Problem (per batch element b of 4):
    q = (Wq @ top_b + bq)      # [32, 4096]
    k = (Wk @ side_b + bk)     # [32, 4096]
    v = (Wv @ side_b + bv)     # [256, 4096]
    E = softmax_over_keys(q.T @ k)        # [4096q, 4096k]
    out_b = top_b + (E @ v.T).T           # [256, 4096]

Sharding: 8 cores = (batch b = core//2) x (query half h = core%2); no
collectives, weights replicated. Each core: 2048 queries x 4096 keys.

Design:
  - Steady state is PE-bound (~2.15us per 4-key-block stage): 16 av
    matmuls (E stationary bf16, [vT|ones] moving 257 cols) + 4 packed
    qk matmuls (K=32 row-tiled).  The exp is off the critical cycle:
    ScalarE exps key blocks 0-2 while the DVE handles block 3 with a
    one-instruction Schraudolph fast-exp (y = x*2^7/ln2 + (16256-c)
    + 1.5*2^23 in fp32; the fp32 add-magic rounds y into the low
    mantissa bits, so the low 16 bits are exactly the bf16 pattern of
    e^x) followed by a DVE int16 compaction of the strided low halves
    so av LDWEIGHTS reads contiguously.
  - av emission lags 3 stages.  The per-chunk epilogue is sliced and
    spread over the next chunk's stages: recip (DVE) + scale
    (ScalarE, per-partition AP) at g=2,3; per-query-block transpose
    (Sync-queue 128x128 DMA) + residual adds (GpSimd tensor_tensor,
    + pre-broadcast bv) at g=4..7.  Chunk 3's transposes split across
    the Sync and ScalarE DMA queues to shorten the tail.
  - Slice-pipelined prologue: side streams in 8 slices; per-slice
    chain = DMA -> bf16 cast (split DVE/ScalarE) -> k-proj (bf16) ->
    k-pack -> 4x v-proj (bf16, one 2-bank PSUM tile, drains split
    DVE/ScalarE).  q projections per chunk: top f16 cast on GpSimd.
    A wk-gated matmul warmup burst trips the HAM clock gate to
    2.4 GHz just before the chains.
  - Row-sum of E rides as a ones column in the av moving operand;
    softmax skips max-subtraction (|scores| < ~50, Schraudolph safe
    to x ~ +88); bv commutes past the normalization into the final
    residual add.  Measured scale-relative absmax 1.28e-2 (gate 2e-2).
"""

import sys

import numpy as np

B, C, H, W = 4, 256, 64, 64
N = H * W      # 4096 keys per batch element
C8 = 32
NCORES = 8
NQ = N // 2    # 2048 queries per core
QC = 512       # query chunk
QB = 128       # query block (matmul M)
KB = 128       # key block
NKB = N // KB  # 32 key blocks
NG = NKB // 4  # 8 groups of 4 packed key blocks
NCHUNK = NQ // QC  # 4

# Schraudolph fast-exp constants (bf16-bits-in-int16 form):
#   v = round(x * 2^7/ln2 + 16256 - c); bf16 bits of e^x ~= v.
#   c = 486411/65536 makes the relative-error sawtooth mean-zero.
#   Adding 1.5*2^23 makes fp32 arithmetic round v into the low 16
#   mantissa bits; bits31..16 are then the constant 0x4B40.
SCHR_A = float(2.0 ** 7 / np.log(2.0))
SCHR_B = float((16256.0 - 486411.0 / 65536.0) + 12582912.0)

_BUILT = {}


def _build(skip_bv):
    for p in ("/opt/trn_rl_repo", "/root/.axon_site/_ro/trn_rl_repo"):
        if p not in sys.path:
            sys.path.append(p)
    import concourse.bass as bass
    import concourse.tile as tile
    from concourse import bacc, mybir

    fp32 = mybir.dt.float32
    f16 = mybir.dt.float16
    bf16 = mybir.dt.bfloat16
    i16 = mybir.dt.int16
    EXP = mybir.ActivationFunctionType.Exp
    ADD = mybir.AluOpType.add
    MULT = mybir.AluOpType.mult

    nc = bacc.Bacc("TRN2", target_bir_lowering=False, debug=False,
                   num_devices=NCORES)

    top_d = nc.dram_tensor("top", [C, NQ], fp32, kind="ExternalInput").ap()
    side_d = nc.dram_tensor("side", [C, N], fp32, kind="ExternalInput").ap()
    wqT_d = nc.dram_tensor("wqT", [C, C8], fp32, kind="ExternalInput").ap()
    wkT_d = nc.dram_tensor("wkT", [C, C8], fp32, kind="ExternalInput").ap()
    wvT_d = nc.dram_tensor("wvT", [C, C], fp32, kind="ExternalInput").ap()
    bq_d = nc.dram_tensor("bq", [C8, 1], fp32, kind="ExternalInput").ap()
    bk_d = nc.dram_tensor("bk", [C8, 1], fp32, kind="ExternalInput").ap()
    bv_d = nc.dram_tensor("bv", [C, 1], fp32, kind="ExternalInput").ap()
    out_d = nc.dram_tensor("out", [C, NQ], fp32, kind="ExternalOutput").ap()

    top_r3 = top_d.rearrange("(t p) n -> p t n", p=128)
    side_r3 = side_d.rearrange("(t p) n -> p t n", p=128)
    wqT_r3 = wqT_d.rearrange("(t p) m -> p t m", p=128)
    wkT_r3 = wkT_d.rearrange("(t p) m -> p t m", p=128)
    wvT_r3 = wvT_d.rearrange("(t p) m -> p t m", p=128)
    bv_r3 = bv_d.rearrange("(t p) o -> p t o", p=128)
    out_r3 = out_d.rearrange("(t p) n -> p t n", p=128)

    with tile.TileContext(nc) as tc:
        with tc.tile_pool(name="persist", bufs=1) as pers, \
             tc.tile_pool(name="work", bufs=1) as work:

            # ---- persistent SBUF tiles ----
            top_sb = pers.tile([128, 2, NQ], fp32, tag="top")
            top_r = pers.tile([128, 2, NQ], f16, tag="top_r")
            side_bf = pers.tile([128, 2, N], bf16, tag="side_bf")
            q_sb = pers.tile([C8, NQ], f16, tag="q")
            q_rep = pers.tile([128, NQ], f16, tag="q_rep")
            k_pack = pers.tile([128, NG, KB], f16, tag="k_pack")
            vT_b = pers.tile([128, NKB, C + 1], bf16, tag="vT")
            out_sb = pers.tile([128, 2, NQ], fp32, tag="out")
            wq_f = pers.tile([128, 2, C8], fp32, tag="wq")
            wk_f = pers.tile([128, 2, C8], fp32, tag="wk")
            wq_h = pers.tile([128, 2, C8], f16, tag="wq_h")
            wk_b = pers.tile([128, 2, C8], bf16, tag="wk_b")
            wv_b = pers.tile([128, 2, C], bf16, tag="wv")
            bq_sb = pers.tile([C8, 1], fp32, tag="bq")
            bk_sb = pers.tile([C8, 1], fp32, tag="bk")
            bv_sb = pers.tile([128, 2, 1], fp32, tag="bv")
            bv128 = pers.tile([128, 2, QB], fp32, tag="bv128")
            bk4 = pers.tile([128, 1], fp32, tag="bk4")
            # i4: horizontal stack of four 32x32 identities; q_rep = i4.T @ q
            i4_r = pers.tile([C8, 128], f16, tag="i4")

            # rowsum ones column of vT (cols 0:C written by v-proj drains)
            nc.gpsimd.memset(vT_b[:, :, C:C + 1], 1.0)

            nc.sync.dma_start(bq_sb[:], bq_d[:])
            nc.sync.dma_start(bk_sb[:], bk_d[:])
            nc.sync.dma_start(bv_sb[:], bv_r3[:])
            for i in range(4):
                nc.vector.tensor_copy(bk4[32 * i:32 * (i + 1), :],
                                      bk_sb[:])
            if not skip_bv:
                nc.gpsimd.memset(bv128[:], 0.0)
                for t in range(2):
                    nc.vector.tensor_scalar_add(bv128[:, t, :],
                                                bv128[:, t, :],
                                                bv_sb[:, t, :])

            with tc.tile_pool(name="stage", bufs=1) as stage, \
                 tc.tile_pool(name="ps_pro", bufs=1, space="PSUM") as psp:

                # exp table preload: first ACTIVATE pays ~2.7us table DMA
                dume = stage.tile([C8, 1], fp32, tag="dume")
                nc.scalar.activation(dume[:], bq_sb[:], EXP)

                # weights
                wv_f = stage.tile([128, 2, C], fp32, tag="wv_f")
                nc.sync.dma_start(wk_f[:], wkT_r3[:])
                nc.vector.tensor_copy(wk_b[:], wk_f[:])
                nc.sync.dma_start(wq_f[:], wqT_r3[:])
                nc.vector.tensor_copy(wq_h[:], wq_f[:])
                nc.sync.dma_start(wv_f[:], wvT_r3[:])
                nc.vector.tensor_copy(wv_b[:], wv_f[:])

                # PE warmup gated on the wk DMA: ~40 matmuls (~2.5us busy)
                # trip the HAM clock gate (4096-cycle activity window)
                # right before the projection chains so they run at
                # 2.4 GHz instead of 1.2.
                pw = psp.tile([C8, 512], fp32, tag="pj", bufs=2, name="pw")
                for wmu in range(40):
                    nc.tensor.matmul(pw[:, 0:C8], wk_b[:, 0, :],
                                     wk_b[:, 1, :], start=True, stop=True)
                pwrd = stage.tile([C8, C8], fp32, tag="pwrd")
                nc.vector.tensor_copy(pwrd[:], pw[:, 0:C8])

                # packing identity for q replication
                i4_f = stage.tile([C8, 128], fp32, tag="i4_f")
                nc.gpsimd.memset(i4_f[:], 0.0)
                nc.gpsimd.affine_select(
                    out=i4_f[:], in_=i4_f[:],
                    compare_op=mybir.AluOpType.not_equal, fill=1.0, base=0,
                    pattern=[[0, 4], [-1, 32]], channel_multiplier=1)
                nc.vector.tensor_copy(i4_r[:], i4_f[:])

                def qprep(qc):
                    # top f16 cast (GpSimd) -> q proj -> +bq -> rows x4
                    qsl = bass.ts(qc, QC)
                    nc.sync.dma_start(top_sb[:, :, qsl], top_r3[:, :, qsl])
                    nc.vector.tensor_copy(top_r[:, 0, qsl],
                                          top_sb[:, 0, qsl])
                    nc.vector.tensor_copy(top_r[:, 1, qsl],
                                          top_sb[:, 1, qsl])
                    pq = psp.tile([C8, 512], fp32, tag="pj", bufs=2,
                                  name=f"pq{qc}")
                    nc.tensor.matmul(pq[:], wq_h[:, 0, :], top_r[:, 0, qsl],
                                     start=True, stop=False)
                    nc.tensor.matmul(pq[:], wq_h[:, 1, :], top_r[:, 1, qsl],
                                     start=False, stop=True)
                    nc.vector.tensor_scalar_add(q_sb[:, qsl], pq[:],
                                                bq_sb[:])
                    pr = psp.tile([128, 512], fp32, tag="big", bufs=3,
                                  name=f"pr{qc}")
                    nc.tensor.matmul(pr[:], i4_r[:], q_sb[:, qsl],
                                     start=True, stop=True)
                    # ScalarE: its PSUM reads are ~4x faster than DVE's
                    nc.scalar.mul(q_rep[:, qsl], pr[:], 1.0)

                # side slice chains: DMA -> bf16 cast (DVE/ScalarE halves)
                # -> k proj (bf16) -> k pack -> v proj per 512-key group
                for s in range(NG):
                    sl = bass.ts(s, 512)
                    sf = stage.tile([128, 2, 512], fp32, tag="sidef",
                                    bufs=NG, name=f"sf{s}")
                    nc.sync.dma_start(sf[:], side_r3[:, :, sl])
                    nc.vector.tensor_copy(side_bf[:, 0, sl], sf[:, 0, :])
                    nc.vector.tensor_copy(side_bf[:, 1, sl], sf[:, 1, :])

                    # col-tiled k proj: output partition group i holds
                    # key sub-block i -> packed layout with no repack
                    pk4 = psp.tile([128, KB], fp32, tag="pp", bufs=2,
                                   name=f"pk{s}")
                    for i in range(4):
                        ksl = slice(512 * s + 128 * i,
                                    512 * s + 128 * (i + 1))
                        nc.tensor.matmul(pk4[32 * i:32 * (i + 1), :],
                                         wk_b[:, 0, :],
                                         side_bf[:, 0, ksl],
                                         start=True, stop=False,
                                         tile_position=(0, 32 * i))
                        nc.tensor.matmul(pk4[32 * i:32 * (i + 1), :],
                                         wk_b[:, 1, :],
                                         side_bf[:, 1, ksl],
                                         start=False, stop=True,
                                         tile_position=(0, 32 * i))
                    nc.vector.tensor_scalar_add(k_pack[:, s, :], pk4[:],
                                                bk4[:])

                    for half in range(2):
                        pv = psp.tile([128, 512], fp32, tag="big", bufs=3,
                                      name=f"pv{s}_{half}")
                        pvv = pv[:].rearrange("p (a b) -> p a b", a=2)
                        for jj in range(2):
                            j = 4 * s + 2 * half + jj
                            jsl = bass.ts(j, KB)
                            nc.tensor.matmul(pvv[:, jj, :],
                                             side_bf[:, 0, jsl],
                                             wv_b[:, 0, :],
                                             start=True, stop=False)
                            nc.tensor.matmul(pvv[:, jj, :],
                                             side_bf[:, 1, jsl],
                                             wv_b[:, 1, :],
                                             start=False, stop=True)
                        j0 = 4 * s + 2 * half
                        nc.scalar.mul(vT_b[:, j0:j0 + 2, 0:C],
                                      pvv[:], 1.0)

                    for _ in range(10):
                        nc.tensor.matmul(pw[:, 0:C8], wk_b[:, 0, :],
                                         wk_b[:, 1, :], start=True,
                                         stop=True)
                    if s == 1:
                        qprep(0)
                    elif s == 3:
                        qprep(1)
                    elif s == 5:
                        qprep(2)
                    elif s == 7:
                        qprep(3)

            # ---- attention ----
            # Stage (chunk qc, group g of 4 key blocks): scores sa (blocks
            # 0-2 -> ScalarE exp) and sb (block 3 -> DVE Schraudolph +
            # int16 compaction).  av matmuls lag 3 stages; the previous
            # chunk's epilogue is spread over stages g=2..7.
            with tc.tile_pool(name="ps_attn", bufs=1, space="PSUM") as psa:
                avs = {}
                scas = {}
                rcs = {}

                def emit_av(st):
                    exa_t, exc_t, qc_t, g_t = st
                    exc_bf = exc_t[:].bitcast(bf16)
                    for i in range(4):
                        j = 4 * g_t + i
                        src = (exa_t[:, i, :] if i < 3 else exc_bf[:])
                        for qb in range(QC // QB):
                            nc.tensor.matmul(
                                avs[qc_t][qb][:],
                                src[:, bass.ts(qb, QB)],
                                vT_b[:, j, :],
                                start=(j == 0), stop=(j == NKB - 1))

                def piece_recip(d, qb):
                    rc = work.tile([128, 1], fp32, tag="rc", bufs=8,
                                   name=f"rc{d}_{qb}")
                    nc.vector.reciprocal(rc[:], avs[d][qb][:, C:C + 1])
                    rcs[(d, qb)] = rc

                def piece_mul(d, qb):
                    # per-query scale on ScalarE (per-partition AP scale)
                    sca = work.tile([128, C], bf16, tag="sca", bufs=8,
                                    name=f"sca{d}_{qb}")
                    nc.scalar.mul(sca[:], avs[d][qb][:, 0:C],
                                  rcs.pop((d, qb))[:])
                    scas[(d, qb)] = sca

                def piece_out(d, qb, dma_eng):
                    # transpose back to [C, q]; residual + bv on GpSimd
                    sca = scas.pop((d, qb))
                    q0 = d * QC + qb * QB
                    for t in range(2):
                        scat = work.tile([128, QB], bf16, tag="scat",
                                         bufs=4, name=f"scat{d}_{qb}{t}")
                        dma_eng.dma_start_transpose(
                            scat[:], sca[:, bass.ts(t, 128)])
                        if skip_bv:
                            nc.gpsimd.tensor_tensor(
                                out_sb[:, t, q0:q0 + QB], scat[:],
                                top_sb[:, t, q0:q0 + QB], op=ADD)
                        else:
                            gt = work.tile([128, QB], fp32, tag="gt",
                                           bufs=4, name=f"gt{d}_{qb}{t}")
                            nc.gpsimd.tensor_tensor(
                                gt[:], scat[:], top_sb[:, t, q0:q0 + QB],
                                op=ADD)
                            nc.gpsimd.tensor_tensor(
                                out_sb[:, t, q0:q0 + QB], gt[:],
                                bv128[:, t, :], op=ADD)

                pending = []
                for qc in range(NCHUNK):
                    qsl = bass.ts(qc, QC)
                    avs[qc] = [psa.tile([128, C + 1], fp32, tag="av",
                                        bufs=4, name=f"av{qc}_{i}")
                               for i in range(QC // QB)]
                    for g in range(NG):
                        sa = psa.tile([128, 3, 512], fp32, tag="sca",
                                      bufs=1, name=f"sa{qc}_{g}")
                        sb = psa.tile([128, 512], fp32, tag="scb",
                                      bufs=1, name=f"sb{qc}_{g}")
                        exa = work.tile([128, 3, 512], bf16, tag="exa",
                                        bufs=5, name=f"exa{qc}_{g}")
                        exb = work.tile([128, 512], fp32, tag="exb",
                                        bufs=5, name=f"exb{qc}_{g}")
                        exc = work.tile([128, 512], i16, tag="exc",
                                        bufs=5, name=f"exc{qc}_{g}")
                        for i in range(3):
                            nc.tensor.matmul(
                                sa[:, i, :],
                                k_pack[32 * i:32 * (i + 1), g, :],
                                q_rep[32 * i:32 * (i + 1), qsl],
                                start=True, stop=True,
                                tile_position=(32 * i, 0))
                        nc.tensor.matmul(
                            sb[:], k_pack[96:128, g, :],
                            q_rep[96:128, qsl],
                            start=True, stop=True, tile_position=(96, 0))
                        d = qc - 1
                        # g==3 pieces must precede the av(qc,0) emission
                        # (their av tiles are being handed over); g==2
                        # pieces must follow the av(d,7) emission.
                        if qc > 0 and g == 3:
                            piece_recip(d, 2)
                            piece_recip(d, 3)
                            piece_mul(d, 2)
                            piece_mul(d, 3)
                        if len(pending) == 3:
                            emit_av(pending.pop(0))
                        if qc > 0:
                            if g == 2:
                                piece_recip(d, 0)
                                piece_recip(d, 1)
                                piece_mul(d, 0)
                                piece_mul(d, 1)
                            elif g in (4, 5, 6, 7):
                                piece_out(d, g - 4, nc.sync)
                                if g == 7:
                                    avs.pop(d)
                                    for t in range(2):
                                        nc.sync.dma_start(
                                            out_r3[:, t, bass.ts(d, QC)],
                                            out_sb[:, t, bass.ts(d, QC)])
                        # exps
                        nc.scalar.activation(exa[:], sa[:], EXP)
                        nc.vector.tensor_scalar(exb[:], sb[:], SCHR_A,
                                                SCHR_B, op0=MULT, op1=ADD)
                        # low 16 bits of each fp32 = bf16 pattern of e^x
                        exbl = exb[:].bitcast(i16).rearrange(
                            "p (q two) -> p two q", two=2)
                        nc.vector.tensor_copy(exc[:], exbl[:, 0, :])
                        pending.append((exa, exc, qc, g))
                # drain last 3 av stages + chunk 3 epilogue (transposes
                # split across the Sync and ScalarE DMA queues)
                for _ in range(3):
                    emit_av(pending.pop(0))
                d = NCHUNK - 1
                for qb in range(4):
                    piece_recip(d, qb)
                for qb in range(4):
                    piece_mul(d, qb)
                for qb in range(4):
                    piece_out(d, qb, nc.sync if qb % 2 == 0 else nc.scalar)
                    q0 = d * QC + qb * QB
                    for t in range(2):
                        nc.sync.dma_start(out_r3[:, t, q0:q0 + QB],
                                          out_sb[:, t, q0:q0 + QB])
                avs.pop(d)

    nc.compile()
    return nc


def _get_built(skip_bv):
    if skip_bv not in _BUILT:
        _BUILT[skip_bv] = _build(skip_bv)
    return _BUILT[skip_bv]


def kernel(topview, sideview, Wq, bq, Wk, bk, Wv, bv):
    from concourse.bass_utils import run_bass_kernel_spmd

    topview = np.asarray(topview, dtype=np.float32)
    sideview = np.asarray(sideview, dtype=np.float32)
    wqT = np.ascontiguousarray(np.asarray(Wq, np.float32).T)
    wkT = np.ascontiguousarray(np.asarray(Wk, np.float32).T)
    wvT = np.ascontiguousarray(np.asarray(Wv, np.float32).T)
    bq = np.asarray(bq, np.float32).reshape(C8, 1)
    bk = np.asarray(bk, np.float32).reshape(C8, 1)
    bv = np.asarray(bv, np.float32).reshape(C, 1)

    top_f = topview.reshape(B, C, N)
    side_f = sideview.reshape(B, C, N)

    in_maps = []
    for core in range(NCORES):
        b, h = core // 2, core % 2
        in_maps.append({
            "top": np.ascontiguousarray(top_f[b, :, h * NQ:(h + 1) * NQ]),
            "side": np.ascontiguousarray(side_f[b]),
            "wqT": wqT, "wkT": wkT, "wvT": wvT,
            "bq": bq, "bk": bk, "bv": bv,
        })

    global _last_in_maps
    _last_in_maps = in_maps

    nc = _get_built(not np.any(np.asarray(bv)))
    res = run_bass_kernel_spmd(nc, in_maps, core_ids=list(range(NCORES)))

    out = np.empty((B, C, N), dtype=np.float32)
    for core in range(NCORES):
        b, h = core // 2, core % 2
        out[b, :, h * NQ:(h + 1) * NQ] = res.results[core]["out"]
    return out.reshape(B, C, H, W)


# revision 24
# speedup vs baseline: 1.0394x; 1.0394x over previous
"""CrossViewTransformer Bass kernel for 8 trn2 NeuronCores (v9).

Problem (per batch element b of 4):
    q = (Wq @ top_b + bq)      # [32, 4096]
    k = (Wk @ side_b + bk)     # [32, 4096]
    v = (Wv @ side_b + bv)     # [256, 4096]
    E = softmax_over_keys(q.T @ k)        # [4096q, 4096k]
    out_b = top_b + (E @ v.T).T           # [256, 4096]

Sharding: 8 cores = (batch b = core//2) x (query half h = core%2); no
collectives, weights replicated. Each core: 2048 queries x 4096 keys.

Design:
  - Steady state is PE-bound (~2.15us per 4-key-block stage): 16 av
    matmuls (E stationary bf16, [vT|ones] moving 257 cols) + 4 packed
    qk matmuls (K=32 row-tiled).  The exp is off the critical cycle:
    ScalarE exps key blocks 0-2 while the DVE handles block 3 with a
    one-instruction Schraudolph fast-exp (y = x*2^7/ln2 + (16256-c)
    + 1.5*2^23 in fp32; the fp32 add-magic rounds y into the low
    mantissa bits, so the low 16 bits are exactly the bf16 pattern of
    e^x) followed by a DVE int16 compaction of the strided low halves
    so av LDWEIGHTS reads contiguously.
  - av emission lags 3 stages.  The per-chunk epilogue is sliced and
    spread over the next chunk's stages: recip (DVE) + scale
    (ScalarE, per-partition AP) at g=2,3; per-query-block transpose
    (Sync-queue 128x128 DMA) + residual adds (GpSimd tensor_tensor,
    + pre-broadcast bv) at g=4..7.  Chunk 3's transposes split across
    the Sync and ScalarE DMA queues to shorten the tail.
  - Slice-pipelined prologue: side streams in 8 slices; per-slice
    chain = DMA -> bf16 cast (split DVE/ScalarE) -> k-proj (bf16) ->
    k-pack -> 4x v-proj (bf16, one 2-bank PSUM tile, drains split
    DVE/ScalarE).  q projections per chunk: top f16 cast on GpSimd.
    A wk-gated matmul warmup burst trips the HAM clock gate to
    2.4 GHz just before the chains.
  - Row-sum of E rides as a ones column in the av moving operand;
    softmax skips max-subtraction (|scores| < ~50, Schraudolph safe
    to x ~ +88); bv commutes past the normalization into the final
    residual add.  Measured scale-relative absmax 1.28e-2 (gate 2e-2).
"""

import sys

import numpy as np

B, C, H, W = 4, 256, 64, 64
N = H * W      # 4096 keys per batch element
C8 = 32
NCORES = 8
NQ = N // 2    # 2048 queries per core
QC = 512       # query chunk
QB = 128       # query block (matmul M)
KB = 128       # key block
NKB = N // KB  # 32 key blocks
NG = NKB // 4  # 8 groups of 4 packed key blocks
NCHUNK = NQ // QC  # 4

# Schraudolph fast-exp constants (bf16-bits-in-int16 form):
#   v = round(x * 2^7/ln2 + 16256 - c); bf16 bits of e^x ~= v.
#   c = 486411/65536 makes the relative-error sawtooth mean-zero.
#   Adding 1.5*2^23 makes fp32 arithmetic round v into the low 16
#   mantissa bits; bits31..16 are then the constant 0x4B40.
SCHR_A = float(2.0 ** 7 / np.log(2.0))
SCHR_B = float((16256.0 - 486411.0 / 65536.0) + 12582912.0)

_BUILT = {}


def _build(skip_bv):
    for p in ("/opt/trn_rl_repo", "/root/.axon_site/_ro/trn_rl_repo"):
        if p not in sys.path:
            sys.path.append(p)
    import concourse.bass as bass
    import concourse.tile as tile
    from concourse import bacc, mybir

    fp32 = mybir.dt.float32
    f16 = mybir.dt.float16
    bf16 = mybir.dt.bfloat16
    i16 = mybir.dt.int16
    EXP = mybir.ActivationFunctionType.Exp
    ADD = mybir.AluOpType.add
    MULT = mybir.AluOpType.mult

    nc = bacc.Bacc("TRN2", target_bir_lowering=False, debug=False,
                   num_devices=NCORES)

    top_d = nc.dram_tensor("top", [C, NQ], fp32, kind="ExternalInput").ap()
    side_d = nc.dram_tensor("side", [C, N], fp32, kind="ExternalInput").ap()
    wqT_d = nc.dram_tensor("wqT", [C, C8], fp32, kind="ExternalInput").ap()
    wkT_d = nc.dram_tensor("wkT", [C, C8], fp32, kind="ExternalInput").ap()
    wvT_d = nc.dram_tensor("wvT", [C, C], fp32, kind="ExternalInput").ap()
    bq_d = nc.dram_tensor("bq", [C8, 1], fp32, kind="ExternalInput").ap()
    bk_d = nc.dram_tensor("bk", [C8, 1], fp32, kind="ExternalInput").ap()
    bv_d = nc.dram_tensor("bv", [C, 1], fp32, kind="ExternalInput").ap()
    out_d = nc.dram_tensor("out", [C, NQ], fp32, kind="ExternalOutput").ap()

    top_r3 = top_d.rearrange("(t p) n -> p t n", p=128)
    side_r3 = side_d.rearrange("(t p) n -> p t n", p=128)
    wqT_r3 = wqT_d.rearrange("(t p) m -> p t m", p=128)
    wkT_r3 = wkT_d.rearrange("(t p) m -> p t m", p=128)
    wvT_r3 = wvT_d.rearrange("(t p) m -> p t m", p=128)
    bv_r3 = bv_d.rearrange("(t p) o -> p t o", p=128)
    out_r3 = out_d.rearrange("(t p) n -> p t n", p=128)

    with tile.TileContext(nc) as tc:
        with tc.tile_pool(name="persist", bufs=1) as pers, \
             tc.tile_pool(name="work", bufs=1) as work:

            # ---- persistent SBUF tiles ----
            top_sb = pers.tile([128, 2, NQ], fp32, tag="top")
            top_r = pers.tile([128, 2, NQ], f16, tag="top_r")
            side_bf = pers.tile([128, 2, N], bf16, tag="side_bf")
            q_sb = pers.tile([C8, NQ], f16, tag="q")
            q_rep = pers.tile([128, NQ], f16, tag="q_rep")
            k_pack = pers.tile([128, NG, KB], f16, tag="k_pack")
            vT_b = pers.tile([128, NKB, C + 1], bf16, tag="vT")
            out_sb = pers.tile([128, 2, NQ], fp32, tag="out")
            wq_f = pers.tile([128, 2, C8], fp32, tag="wq")
            wk_f = pers.tile([128, 2, C8], fp32, tag="wk")
            wq_h = pers.tile([128, 2, C8], f16, tag="wq_h")
            wk_b = pers.tile([128, 2, C8], bf16, tag="wk_b")
            wv_b = pers.tile([128, 2, C], bf16, tag="wv")
            bq_sb = pers.tile([C8, 1], fp32, tag="bq")
            bk_sb = pers.tile([C8, 1], fp32, tag="bk")
            bv_sb = pers.tile([128, 2, 1], fp32, tag="bv")
            bv128 = pers.tile([128, 2, QB], fp32, tag="bv128")
            bk4 = pers.tile([128, 1], fp32, tag="bk4")
            # i4: horizontal stack of four 32x32 identities; q_rep = i4.T @ q
            i4_r = pers.tile([C8, 128], f16, tag="i4")

            # rowsum ones column of vT (cols 0:C written by v-proj drains)
            nc.gpsimd.memset(vT_b[:, :, C:C + 1], 1.0)

            nc.sync.dma_start(bq_sb[:], bq_d[:])
            nc.sync.dma_start(bk_sb[:], bk_d[:])
            nc.sync.dma_start(bv_sb[:], bv_r3[:])
            for i in range(4):
                nc.vector.tensor_copy(bk4[32 * i:32 * (i + 1), :],
                                      bk_sb[:])
            if not skip_bv:
                nc.gpsimd.memset(bv128[:], 0.0)
                for t in range(2):
                    nc.vector.tensor_scalar_add(bv128[:, t, :],
                                                bv128[:, t, :],
                                                bv_sb[:, t, :])

            with tc.tile_pool(name="stage", bufs=1) as stage, \
                 tc.tile_pool(name="ps_pro", bufs=1, space="PSUM") as psp:

                # exp table preload: first ACTIVATE pays ~2.7us table DMA
                dume = stage.tile([C8, 1], fp32, tag="dume")
                nc.scalar.activation(dume[:], bq_sb[:], EXP)

                # weights
                wv_f = stage.tile([128, 2, C], fp32, tag="wv_f")
                nc.sync.dma_start(wk_f[:], wkT_r3[:])
                nc.vector.tensor_copy(wk_b[:], wk_f[:])
                nc.sync.dma_start(wq_f[:], wqT_r3[:])
                nc.vector.tensor_copy(wq_h[:], wq_f[:])
                nc.sync.dma_start(wv_f[:], wvT_r3[:])
                nc.vector.tensor_copy(wv_b[:], wv_f[:])

                # PE warmup gated on the wk DMA: ~40 matmuls (~2.5us busy)
                # trip the HAM clock gate (4096-cycle activity window)
                # right before the projection chains so they run at
                # 2.4 GHz instead of 1.2.
                pw = psp.tile([C8, 512], fp32, tag="pj", bufs=2, name="pw")
                for wmu in range(40):
                    nc.tensor.matmul(pw[:, 0:C8], wk_b[:, 0, :],
                                     wk_b[:, 1, :], start=True, stop=True)
                pwrd = stage.tile([C8, C8], fp32, tag="pwrd")
                nc.vector.tensor_copy(pwrd[:], pw[:, 0:C8])

                # packing identity for q replication
                i4_f = stage.tile([C8, 128], fp32, tag="i4_f")
                nc.gpsimd.memset(i4_f[:], 0.0)
                nc.gpsimd.affine_select(
                    out=i4_f[:], in_=i4_f[:],
                    compare_op=mybir.AluOpType.not_equal, fill=1.0, base=0,
                    pattern=[[0, 4], [-1, 32]], channel_multiplier=1)
                nc.vector.tensor_copy(i4_r[:], i4_f[:])

                def qprep(qc):
                    # top f16 cast (GpSimd) -> q proj -> +bq -> rows x4
                    qsl = bass.ts(qc, QC)
                    nc.sync.dma_start(top_sb[:, :, qsl], top_r3[:, :, qsl])
                    nc.vector.tensor_copy(top_r[:, 0, qsl],
                                          top_sb[:, 0, qsl])
                    nc.vector.tensor_copy(top_r[:, 1, qsl],
                                          top_sb[:, 1, qsl])
                    pq = psp.tile([C8, 512], fp32, tag="pj", bufs=2,
                                  name=f"pq{qc}")
                    nc.tensor.matmul(pq[:], wq_h[:, 0, :], top_r[:, 0, qsl],
                                     start=True, stop=False)
                    nc.tensor.matmul(pq[:], wq_h[:, 1, :], top_r[:, 1, qsl],
                                     start=False, stop=True)
                    nc.vector.tensor_scalar_add(q_sb[:, qsl], pq[:],
                                                bq_sb[:])
                    pr = psp.tile([128, 512], fp32, tag="big", bufs=3,
                                  name=f"pr{qc}")
                    nc.tensor.matmul(pr[:], i4_r[:], q_sb[:, qsl],
                                     start=True, stop=True)
                    # ScalarE: its PSUM reads are ~4x faster than DVE's
                    nc.scalar.mul(q_rep[:, qsl], pr[:], 1.0)

                # side slice chains: DMA -> bf16 cast (DVE/ScalarE halves)
                # -> k proj (bf16) -> k pack -> v proj per 512-key group
                for s in range(NG):
                    sl = bass.ts(s, 512)
                    sf = stage.tile([128, 2, 512], fp32, tag="sidef",
                                    bufs=NG, name=f"sf{s}")
                    nc.sync.dma_start(sf[:], side_r3[:, :, sl])
                    nc.vector.tensor_copy(side_bf[:, 0, sl], sf[:, 0, :])
                    nc.vector.tensor_copy(side_bf[:, 1, sl], sf[:, 1, :])

                    # col-tiled k proj: output partition group i holds
                    # key sub-block i -> packed layout with no repack
                    pk4 = psp.tile([128, KB], fp32, tag="pp", bufs=2,
                                   name=f"pk{s}")
                    for i in range(4):
                        ksl = slice(512 * s + 128 * i,
                                    512 * s + 128 * (i + 1))
                        nc.tensor.matmul(pk4[32 * i:32 * (i + 1), :],
                                         wk_b[:, 0, :],
                                         side_bf[:, 0, ksl],
                                         start=True, stop=False,
                                         tile_position=(0, 32 * i))
                        nc.tensor.matmul(pk4[32 * i:32 * (i + 1), :],
                                         wk_b[:, 1, :],
                                         side_bf[:, 1, ksl],
                                         start=False, stop=True,
                                         tile_position=(0, 32 * i))
                    nc.vector.tensor_scalar_add(k_pack[:, s, :], pk4[:],
                                                bk4[:])

                    for half in range(2):
                        pv = psp.tile([128, 512], fp32, tag="big", bufs=3,
                                      name=f"pv{s}_{half}")
                        pvv = pv[:].rearrange("p (a b) -> p a b", a=2)
                        for jj in range(2):
                            j = 4 * s + 2 * half + jj
                            jsl = bass.ts(j, KB)
                            nc.tensor.matmul(pvv[:, jj, :],
                                             side_bf[:, 0, jsl],
                                             wv_b[:, 0, :],
                                             start=True, stop=False)
                            nc.tensor.matmul(pvv[:, jj, :],
                                             side_bf[:, 1, jsl],
                                             wv_b[:, 1, :],
                                             start=False, stop=True)
                        j0 = 4 * s + 2 * half
                        nc.scalar.mul(vT_b[:, j0:j0 + 2, 0:C],
                                      pvv[:], 1.0)

                    for _ in range(10):
                        nc.tensor.matmul(pw[:, 0:C8], wk_b[:, 0, :],
                                         wk_b[:, 1, :], start=True,
                                         stop=True)
                    if s == 1:
                        qprep(0)
                    elif s == 3:
                        qprep(1)
                    elif s == 5:
                        qprep(2)
                    elif s == 7:
                        qprep(3)

            # ---- attention ----
            # Stage (chunk qc, group g of 4 key blocks): scores sa (blocks
            # 0-2 -> ScalarE exp) and sb (block 3 -> DVE Schraudolph +
            # int16 compaction).  av matmuls lag 3 stages; the previous
            # chunk's epilogue is spread over stages g=2..7.
            with tc.tile_pool(name="ps_attn", bufs=1, space="PSUM") as psa:
                avs = {}
                scas = {}
                rcs = {}

                def emit_av(st):
                    exa_t, exc_t, qc_t, g_t = st
                    exc_bf = exc_t[:].bitcast(bf16)
                    for i in range(4):
                        j = 4 * g_t + i
                        src = (exa_t[:, i, :] if i < 3 else exc_bf[:])
                        for qb in range(QC // QB):
                            nc.tensor.matmul(
                                avs[qc_t][qb][:],
                                src[:, bass.ts(qb, QB)],
                                vT_b[:, j, :],
                                start=(j == 0), stop=(j == NKB - 1))

                def piece_recip(d, qb):
                    rc = work.tile([128, 1], fp32, tag="rc", bufs=8,
                                   name=f"rc{d}_{qb}")
                    nc.vector.reciprocal(rc[:], avs[d][qb][:, C:C + 1])
                    rcs[(d, qb)] = rc

                def piece_mul(d, qb):
                    # per-query scale on ScalarE (per-partition AP scale)
                    sca = work.tile([128, C], bf16, tag="sca", bufs=8,
                                    name=f"sca{d}_{qb}")
                    nc.scalar.mul(sca[:], avs[d][qb][:, 0:C],
                                  rcs.pop((d, qb))[:])
                    scas[(d, qb)] = sca

                def piece_out(d, qb, dma_eng):
                    # transpose back to [C, q]; residual + bv on GpSimd
                    sca = scas.pop((d, qb))
                    q0 = d * QC + qb * QB
                    for t in range(2):
                        scat = work.tile([128, QB], bf16, tag="scat",
                                         bufs=4, name=f"scat{d}_{qb}{t}")
                        dma_eng.dma_start_transpose(
                            scat[:], sca[:, bass.ts(t, 128)])
                        if skip_bv:
                            nc.gpsimd.tensor_tensor(
                                out_sb[:, t, q0:q0 + QB], scat[:],
                                top_sb[:, t, q0:q0 + QB], op=ADD)
                        else:
                            gt = work.tile([128, QB], fp32, tag="gt",
                                           bufs=4, name=f"gt{d}_{qb}{t}")
                            nc.gpsimd.tensor_tensor(
                                gt[:], scat[:], top_sb[:, t, q0:q0 + QB],
                                op=ADD)
                            nc.gpsimd.tensor_tensor(
                                out_sb[:, t, q0:q0 + QB], gt[:],
                                bv128[:, t, :], op=ADD)

                pending = []
                for qc in range(NCHUNK):
                    qsl = bass.ts(qc, QC)
                    avs[qc] = [psa.tile([128, C + 1], fp32, tag="av",
                                        bufs=4, name=f"av{qc}_{i}")
                               for i in range(QC // QB)]
                    for g in range(NG):
                        sa = psa.tile([128, 3, 512], fp32, tag="sca",
                                      bufs=1, name=f"sa{qc}_{g}")
                        sb = psa.tile([128, 512], fp32, tag="scb",
                                      bufs=1, name=f"sb{qc}_{g}")
                        exa = work.tile([128, 3, 512], bf16, tag="exa",
                                        bufs=5, name=f"exa{qc}_{g}")
                        exb = work.tile([128, 512], fp32, tag="exb",
                                        bufs=5, name=f"exb{qc}_{g}")
                        exc = work.tile([128, 512], i16, tag="exc",
                                        bufs=5, name=f"exc{qc}_{g}")
                        for i in range(3):
                            nc.tensor.matmul(
                                sa[:, i, :],
                                k_pack[32 * i:32 * (i + 1), g, :],
                                q_rep[32 * i:32 * (i + 1), qsl],
                                start=True, stop=True,
                                tile_position=(32 * i, 0))
                        nc.tensor.matmul(
                            sb[:], k_pack[96:128, g, :],
                            q_rep[96:128, qsl],
                            start=True, stop=True, tile_position=(96, 0))
                        d = qc - 1
                        # g==3 pieces must precede the av(qc,0) emission
                        # (their av tiles are being handed over); g==2
                        # pieces must follow the av(d,7) emission.
                        if qc > 0 and g == 3:
                            piece_recip(d, 2)
                            piece_recip(d, 3)
                            piece_mul(d, 2)
                            piece_mul(d, 3)
                        if len(pending) == 3:
                            emit_av(pending.pop(0))
                        if qc > 0:
                            if g == 2:
                                piece_recip(d, 0)
                                piece_recip(d, 1)
                                piece_mul(d, 0)
                                piece_mul(d, 1)
                            elif g in (4, 5, 6, 7):
                                piece_out(d, g - 4, nc.sync)
                                if g == 7:
                                    avs.pop(d)
                                    for t in range(2):
                                        nc.sync.dma_start(
                                            out_r3[:, t, bass.ts(d, QC)],
                                            out_sb[:, t, bass.ts(d, QC)])
                        # exps
                        nc.scalar.activation(exa[:], sa[:], EXP)
                        nc.vector.tensor_scalar(exb[:], sb[:], SCHR_A,
                                                SCHR_B, op0=MULT, op1=ADD)
                        # low 16 bits of each fp32 = bf16 pattern of e^x
                        exbl = exb[:].bitcast(i16).rearrange(
                            "p (q two) -> p two q", two=2)
                        nc.vector.tensor_copy(exc[:], exbl[:, 0, :])
                        pending.append((exa, exc, qc, g))
                # drain last 3 av stages + chunk 3 epilogue (transposes
                # split across the Sync and ScalarE DMA queues)
                for _ in range(3):
                    emit_av(pending.pop(0))
                d = NCHUNK - 1
                for qb in range(4):
                    piece_recip(d, qb)
                for qb in range(4):
                    piece_mul(d, qb)
                for qb in range(4):
                    piece_out(d, qb, nc.sync if qb % 2 == 0 else nc.scalar)
                avs.pop(d)
                for t in range(2):
                    nc.sync.dma_start(out_r3[:, t, bass.ts(d, QC)],
                                      out_sb[:, t, bass.ts(d, QC)])

    nc.compile()
    return nc


def _get_built(skip_bv):
    if skip_bv not in _BUILT:
        _BUILT[skip_bv] = _build(skip_bv)
    return _BUILT[skip_bv]


def kernel(topview, sideview, Wq, bq, Wk, bk, Wv, bv):
    from concourse.bass_utils import run_bass_kernel_spmd

    topview = np.asarray(topview, dtype=np.float32)
    sideview = np.asarray(sideview, dtype=np.float32)
    wqT = np.ascontiguousarray(np.asarray(Wq, np.float32).T)
    wkT = np.ascontiguousarray(np.asarray(Wk, np.float32).T)
    wvT = np.ascontiguousarray(np.asarray(Wv, np.float32).T)
    bq = np.asarray(bq, np.float32).reshape(C8, 1)
    bk = np.asarray(bk, np.float32).reshape(C8, 1)
    bv = np.asarray(bv, np.float32).reshape(C, 1)

    top_f = topview.reshape(B, C, N)
    side_f = sideview.reshape(B, C, N)

    in_maps = []
    for core in range(NCORES):
        b, h = core // 2, core % 2
        in_maps.append({
            "top": np.ascontiguousarray(top_f[b, :, h * NQ:(h + 1) * NQ]),
            "side": np.ascontiguousarray(side_f[b]),
            "wqT": wqT, "wkT": wkT, "wvT": wvT,
            "bq": bq, "bk": bk, "bv": bv,
        })

    global _last_in_maps
    _last_in_maps = in_maps

    nc = _get_built(not np.any(np.asarray(bv)))
    res = run_bass_kernel_spmd(nc, in_maps, core_ids=list(range(NCORES)))

    out = np.empty((B, C, N), dtype=np.float32)
    for core in range(NCORES):
        b, h = core // 2, core % 2
        out[b, :, h * NQ:(h + 1) * NQ] = res.results[core]["out"]
    return out.reshape(B, C, H, W)


# revision 25
# speedup vs baseline: 1.0576x; 1.0175x over previous
"""CrossViewTransformer Bass kernel for 8 trn2 NeuronCores (v9).

Problem (per batch element b of 4):
    q = (Wq @ top_b + bq)      # [32, 4096]
    k = (Wk @ side_b + bk)     # [32, 4096]
    v = (Wv @ side_b + bv)     # [256, 4096]
    E = softmax_over_keys(q.T @ k)        # [4096q, 4096k]
    out_b = top_b + (E @ v.T).T           # [256, 4096]

Sharding: 8 cores = (batch b = core//2) x (query half h = core%2); no
collectives, weights replicated. Each core: 2048 queries x 4096 keys.

Design:
  - Steady state is PE-bound (~2.15us per 4-key-block stage): 16 av
    matmuls (E stationary bf16, [vT|ones] moving 257 cols) + 4 packed
    qk matmuls (K=32 row-tiled).  The exp is off the critical cycle:
    ScalarE exps key blocks 0-2 while the DVE handles block 3 with a
    one-instruction Schraudolph fast-exp (y = x*2^7/ln2 + (16256-c)
    + 1.5*2^23 in fp32; the fp32 add-magic rounds y into the low
    mantissa bits, so the low 16 bits are exactly the bf16 pattern of
    e^x) followed by a DVE int16 compaction of the strided low halves
    so av LDWEIGHTS reads contiguously.
  - av emission lags 3 stages.  The per-chunk epilogue is sliced and
    spread over the next chunk's stages: recip (DVE) + scale
    (ScalarE, per-partition AP) at g=2,3; per-query-block transpose
    (Sync-queue 128x128 DMA) + residual adds (GpSimd tensor_tensor,
    + pre-broadcast bv) at g=4..7.  Chunk 3's transposes split across
    the Sync and ScalarE DMA queues to shorten the tail.
  - Slice-pipelined prologue: side streams in 8 slices; per-slice
    chain = DMA -> bf16 cast (split DVE/ScalarE) -> k-proj (bf16) ->
    k-pack -> 4x v-proj (bf16, one 2-bank PSUM tile, drains split
    DVE/ScalarE).  q projections per chunk: top f16 cast on GpSimd.
    A wk-gated matmul warmup burst trips the HAM clock gate to
    2.4 GHz just before the chains.
  - Row-sum of E rides as a ones column in the av moving operand;
    softmax skips max-subtraction (|scores| < ~50, Schraudolph safe
    to x ~ +88); bv commutes past the normalization into the final
    residual add.  Measured scale-relative absmax 1.28e-2 (gate 2e-2).
"""

import sys

import numpy as np

B, C, H, W = 4, 256, 64, 64
N = H * W      # 4096 keys per batch element
C8 = 32
NCORES = 8
NQ = N // 2    # 2048 queries per core
QC = 512       # query chunk
QB = 128       # query block (matmul M)
KB = 128       # key block
NKB = N // KB  # 32 key blocks
NG = NKB // 4  # 8 groups of 4 packed key blocks
NCHUNK = NQ // QC  # 4

# Schraudolph fast-exp constants (bf16-bits-in-int16 form):
#   v = round(x * 2^7/ln2 + 16256 - c); bf16 bits of e^x ~= v.
#   c = 486411/65536 makes the relative-error sawtooth mean-zero.
#   Adding 1.5*2^23 makes fp32 arithmetic round v into the low 16
#   mantissa bits; bits31..16 are then the constant 0x4B40.
SCHR_A = float(2.0 ** 7 / np.log(2.0))
SCHR_B = float((16256.0 - 486411.0 / 65536.0) + 12582912.0)

_BUILT = {}


def _build(skip_bv):
    for p in ("/opt/trn_rl_repo", "/root/.axon_site/_ro/trn_rl_repo"):
        if p not in sys.path:
            sys.path.append(p)
    import concourse.bass as bass
    import concourse.tile as tile
    from concourse import bacc, mybir

    fp32 = mybir.dt.float32
    f16 = mybir.dt.float16
    bf16 = mybir.dt.bfloat16
    i16 = mybir.dt.int16
    EXP = mybir.ActivationFunctionType.Exp
    ADD = mybir.AluOpType.add
    MULT = mybir.AluOpType.mult

    nc = bacc.Bacc("TRN2", target_bir_lowering=False, debug=False,
                   num_devices=NCORES)

    top_d = nc.dram_tensor("top", [C, NQ], fp32, kind="ExternalInput").ap()
    side_d = nc.dram_tensor("side", [C, N], fp32, kind="ExternalInput").ap()
    wqT_d = nc.dram_tensor("wqT", [C, C8], fp32, kind="ExternalInput").ap()
    wkT_d = nc.dram_tensor("wkT", [C, C8], fp32, kind="ExternalInput").ap()
    wvT_d = nc.dram_tensor("wvT", [C, C], fp32, kind="ExternalInput").ap()
    bq_d = nc.dram_tensor("bq", [C8, 1], fp32, kind="ExternalInput").ap()
    bk_d = nc.dram_tensor("bk", [C8, 1], fp32, kind="ExternalInput").ap()
    bv_d = nc.dram_tensor("bv", [C, 1], fp32, kind="ExternalInput").ap()
    out_d = nc.dram_tensor("out", [C, NQ], fp32, kind="ExternalOutput").ap()

    top_r3 = top_d.rearrange("(t p) n -> p t n", p=128)
    side_r3 = side_d.rearrange("(t p) n -> p t n", p=128)
    wqT_r3 = wqT_d.rearrange("(t p) m -> p t m", p=128)
    wkT_r3 = wkT_d.rearrange("(t p) m -> p t m", p=128)
    wvT_r3 = wvT_d.rearrange("(t p) m -> p t m", p=128)
    bv_r3 = bv_d.rearrange("(t p) o -> p t o", p=128)
    out_r3 = out_d.rearrange("(t p) n -> p t n", p=128)

    with tile.TileContext(nc) as tc:
        with tc.tile_pool(name="persist", bufs=1) as pers, \
             tc.tile_pool(name="work", bufs=1) as work:

            # ---- persistent SBUF tiles ----
            top_sb = pers.tile([128, 2, NQ], fp32, tag="top")
            top_r = pers.tile([128, 2, NQ], f16, tag="top_r")
            side_bf = pers.tile([128, 2, N], bf16, tag="side_bf")
            q_sb = pers.tile([C8, NQ], f16, tag="q")
            q_rep = pers.tile([128, NQ], f16, tag="q_rep")
            k_pack = pers.tile([128, NG, KB], f16, tag="k_pack")
            vT_b = pers.tile([128, NKB, C + 1], bf16, tag="vT")
            out_sb = pers.tile([128, 2, NQ], fp32, tag="out")
            wq_f = pers.tile([128, 2, C8], fp32, tag="wq")
            wk_f = pers.tile([128, 2, C8], fp32, tag="wk")
            wq_h = pers.tile([128, 2, C8], f16, tag="wq_h")
            wk_b = pers.tile([128, 2, C8], bf16, tag="wk_b")
            wv_b = pers.tile([128, 2, C], bf16, tag="wv")
            bq_sb = pers.tile([C8, 1], fp32, tag="bq")
            bk_sb = pers.tile([C8, 1], fp32, tag="bk")
            bv_sb = pers.tile([128, 2, 1], fp32, tag="bv")
            bv128 = pers.tile([128, 2, QB], fp32, tag="bv128")
            bk4 = pers.tile([128, 1], fp32, tag="bk4")
            # i4: horizontal stack of four 32x32 identities; q_rep = i4.T @ q
            i4_r = pers.tile([C8, 128], f16, tag="i4")

            # rowsum ones column of vT (cols 0:C written by v-proj drains)
            nc.gpsimd.memset(vT_b[:, :, C:C + 1], 1.0)

            nc.sync.dma_start(bq_sb[:], bq_d[:])
            nc.sync.dma_start(bk_sb[:], bk_d[:])
            nc.sync.dma_start(bv_sb[:], bv_r3[:])
            for i in range(4):
                nc.vector.tensor_copy(bk4[32 * i:32 * (i + 1), :],
                                      bk_sb[:])
            if not skip_bv:
                nc.gpsimd.memset(bv128[:], 0.0)
                for t in range(2):
                    nc.vector.tensor_scalar_add(bv128[:, t, :],
                                                bv128[:, t, :],
                                                bv_sb[:, t, :])

            with tc.tile_pool(name="stage", bufs=1) as stage, \
                 tc.tile_pool(name="ps_pro", bufs=1, space="PSUM") as psp:

                # exp table preload: first ACTIVATE pays ~2.7us table DMA
                dume = stage.tile([C8, 1], fp32, tag="dume")
                nc.scalar.activation(dume[:], bq_sb[:], EXP)

                # weights
                wv_f = stage.tile([128, 2, C], fp32, tag="wv_f")
                nc.sync.dma_start(wk_f[:], wkT_r3[:])
                nc.vector.tensor_copy(wk_b[:], wk_f[:])
                nc.sync.dma_start(wq_f[:], wqT_r3[:])
                nc.vector.tensor_copy(wq_h[:], wq_f[:])
                nc.sync.dma_start(wv_f[:], wvT_r3[:])
                nc.vector.tensor_copy(wv_b[:], wv_f[:])

                sfh = []
                for h in range(2):
                    t_ = stage.tile([128, 2, NQ], fp32, tag=f"sfh{h}",
                                    bufs=1, name=f"sfh{h}")
                    sfh.append(t_)
                nc.sync.dma_start(sfh[0][:], side_r3[:, :, 0:NQ])
                nc.sync.dma_start(top_sb[:], top_r3[:])
                nc.sync.dma_start(sfh[1][:], side_r3[:, :, NQ:N])

                # PE warmup gated on the wk DMA: ~40 matmuls (~2.5us busy)
                # trip the HAM clock gate (4096-cycle activity window)
                # right before the projection chains so they run at
                # 2.4 GHz instead of 1.2.
                pw = psp.tile([C8, 512], fp32, tag="pj", bufs=2, name="pw")
                for wmu in range(40):
                    nc.tensor.matmul(pw[:, 0:C8], wk_b[:, 0, :],
                                     wk_b[:, 1, :], start=True, stop=True)
                pwrd = stage.tile([C8, C8], fp32, tag="pwrd")
                nc.vector.tensor_copy(pwrd[:], pw[:, 0:C8])

                # packing identity for q replication
                i4_f = stage.tile([C8, 128], fp32, tag="i4_f")
                nc.gpsimd.memset(i4_f[:], 0.0)
                nc.gpsimd.affine_select(
                    out=i4_f[:], in_=i4_f[:],
                    compare_op=mybir.AluOpType.not_equal, fill=1.0, base=0,
                    pattern=[[0, 4], [-1, 32]], channel_multiplier=1)
                nc.vector.tensor_copy(i4_r[:], i4_f[:])

                def qprep(qc):
                    # top f16 cast (GpSimd) -> q proj -> +bq -> rows x4
                    qsl = bass.ts(qc, QC)
                    nc.vector.tensor_copy(top_r[:, 0, qsl],
                                          top_sb[:, 0, qsl])
                    nc.vector.tensor_copy(top_r[:, 1, qsl],
                                          top_sb[:, 1, qsl])
                    pq = psp.tile([C8, 512], fp32, tag="pj", bufs=2,
                                  name=f"pq{qc}")
                    nc.tensor.matmul(pq[:], wq_h[:, 0, :], top_r[:, 0, qsl],
                                     start=True, stop=False)
                    nc.tensor.matmul(pq[:], wq_h[:, 1, :], top_r[:, 1, qsl],
                                     start=False, stop=True)
                    nc.vector.tensor_scalar_add(q_sb[:, qsl], pq[:],
                                                bq_sb[:])
                    pr = psp.tile([128, 512], fp32, tag="big", bufs=3,
                                  name=f"pr{qc}")
                    nc.tensor.matmul(pr[:], i4_r[:], q_sb[:, qsl],
                                     start=True, stop=True)
                    # ScalarE: its PSUM reads are ~4x faster than DVE's
                    nc.scalar.mul(q_rep[:, qsl], pr[:], 1.0)

                # side slice chains: DMA -> bf16 cast (DVE/ScalarE halves)
                # -> k proj (bf16) -> k pack -> v proj per 512-key group
                for s in range(NG):
                    sl = bass.ts(s, 512)
                    lsl = bass.ts(s % 4, 512)
                    sf = sfh[s // 4]
                    nc.vector.tensor_copy(side_bf[:, 0, sl],
                                          sf[:, 0, lsl])
                    nc.vector.tensor_copy(side_bf[:, 1, sl],
                                          sf[:, 1, lsl])

                    # col-tiled k proj: output partition group i holds
                    # key sub-block i -> packed layout with no repack
                    pk4 = psp.tile([128, KB], fp32, tag="pp", bufs=2,
                                   name=f"pk{s}")
                    for i in range(4):
                        ksl = slice(512 * s + 128 * i,
                                    512 * s + 128 * (i + 1))
                        nc.tensor.matmul(pk4[32 * i:32 * (i + 1), :],
                                         wk_b[:, 0, :],
                                         side_bf[:, 0, ksl],
                                         start=True, stop=False,
                                         tile_position=(0, 32 * i))
                        nc.tensor.matmul(pk4[32 * i:32 * (i + 1), :],
                                         wk_b[:, 1, :],
                                         side_bf[:, 1, ksl],
                                         start=False, stop=True,
                                         tile_position=(0, 32 * i))
                    nc.vector.tensor_scalar_add(k_pack[:, s, :], pk4[:],
                                                bk4[:])

                    for half in range(2):
                        pv = psp.tile([128, 512], fp32, tag="big", bufs=3,
                                      name=f"pv{s}_{half}")
                        pvv = pv[:].rearrange("p (a b) -> p a b", a=2)
                        for jj in range(2):
                            j = 4 * s + 2 * half + jj
                            jsl = bass.ts(j, KB)
                            nc.tensor.matmul(pvv[:, jj, :],
                                             side_bf[:, 0, jsl],
                                             wv_b[:, 0, :],
                                             start=True, stop=False)
                            nc.tensor.matmul(pvv[:, jj, :],
                                             side_bf[:, 1, jsl],
                                             wv_b[:, 1, :],
                                             start=False, stop=True)
                        j0 = 4 * s + 2 * half
                        nc.scalar.mul(vT_b[:, j0:j0 + 2, 0:C],
                                      pvv[:], 1.0)

                    for _ in range(10):
                        nc.tensor.matmul(pw[:, 0:C8], wk_b[:, 0, :],
                                         wk_b[:, 1, :], start=True,
                                         stop=True)
                    if s == 1:
                        qprep(0)
                    elif s == 3:
                        qprep(1)
                    elif s == 5:
                        qprep(2)
                    elif s == 7:
                        qprep(3)

            # ---- attention ----
            # Stage (chunk qc, group g of 4 key blocks): scores sa (blocks
            # 0-2 -> ScalarE exp) and sb (block 3 -> DVE Schraudolph +
            # int16 compaction).  av matmuls lag 3 stages; the previous
            # chunk's epilogue is spread over stages g=2..7.
            with tc.tile_pool(name="ps_attn", bufs=1, space="PSUM") as psa:
                avs = {}
                scas = {}
                rcs = {}

                def emit_av(st):
                    exa_t, exc_t, qc_t, g_t = st
                    exc_bf = exc_t[:].bitcast(bf16)
                    for i in range(4):
                        j = 4 * g_t + i
                        src = (exa_t[:, i, :] if i < 3 else exc_bf[:])
                        for qb in range(QC // QB):
                            nc.tensor.matmul(
                                avs[qc_t][qb][:],
                                src[:, bass.ts(qb, QB)],
                                vT_b[:, j, :],
                                start=(j == 0), stop=(j == NKB - 1))

                def piece_recip(d, qb):
                    rc = work.tile([128, 1], fp32, tag="rc", bufs=8,
                                   name=f"rc{d}_{qb}")
                    nc.vector.reciprocal(rc[:], avs[d][qb][:, C:C + 1])
                    rcs[(d, qb)] = rc

                def piece_mul(d, qb):
                    # per-query scale on ScalarE (per-partition AP scale)
                    sca = work.tile([128, C], bf16, tag="sca", bufs=8,
                                    name=f"sca{d}_{qb}")
                    nc.scalar.mul(sca[:], avs[d][qb][:, 0:C],
                                  rcs.pop((d, qb))[:])
                    scas[(d, qb)] = sca

                def piece_out(d, qb, dma_eng):
                    # transpose back to [C, q]; residual + bv on GpSimd
                    sca = scas.pop((d, qb))
                    q0 = d * QC + qb * QB
                    for t in range(2):
                        scat = work.tile([128, QB], bf16, tag="scat",
                                         bufs=4, name=f"scat{d}_{qb}{t}")
                        dma_eng.dma_start_transpose(
                            scat[:], sca[:, bass.ts(t, 128)])
                        if skip_bv:
                            nc.gpsimd.tensor_tensor(
                                out_sb[:, t, q0:q0 + QB], scat[:],
                                top_sb[:, t, q0:q0 + QB], op=ADD)
                        else:
                            gt = work.tile([128, QB], fp32, tag="gt",
                                           bufs=4, name=f"gt{d}_{qb}{t}")
                            nc.gpsimd.tensor_tensor(
                                gt[:], scat[:], top_sb[:, t, q0:q0 + QB],
                                op=ADD)
                            nc.gpsimd.tensor_tensor(
                                out_sb[:, t, q0:q0 + QB], gt[:],
                                bv128[:, t, :], op=ADD)

                pending = []
                for qc in range(NCHUNK):
                    qsl = bass.ts(qc, QC)
                    avs[qc] = [psa.tile([128, C + 1], fp32, tag="av",
                                        bufs=4, name=f"av{qc}_{i}")
                               for i in range(QC // QB)]
                    for g in range(NG):
                        sa = psa.tile([128, 3, 512], fp32, tag="sca",
                                      bufs=1, name=f"sa{qc}_{g}")
                        sb = psa.tile([128, 512], fp32, tag="scb",
                                      bufs=1, name=f"sb{qc}_{g}")
                        exa = work.tile([128, 3, 512], bf16, tag="exa",
                                        bufs=5, name=f"exa{qc}_{g}")
                        exb = work.tile([128, 512], fp32, tag="exb",
                                        bufs=5, name=f"exb{qc}_{g}")
                        exc = work.tile([128, 512], i16, tag="exc",
                                        bufs=5, name=f"exc{qc}_{g}")
                        for i in range(3):
                            nc.tensor.matmul(
                                sa[:, i, :],
                                k_pack[32 * i:32 * (i + 1), g, :],
                                q_rep[32 * i:32 * (i + 1), qsl],
                                start=True, stop=True,
                                tile_position=(32 * i, 0))
                        nc.tensor.matmul(
                            sb[:], k_pack[96:128, g, :],
                            q_rep[96:128, qsl],
                            start=True, stop=True, tile_position=(96, 0))
                        d = qc - 1
                        # g==3 pieces must precede the av(qc,0) emission
                        # (their av tiles are being handed over); g==2
                        # pieces must follow the av(d,7) emission.
                        if qc > 0 and g == 3:
                            piece_recip(d, 2)
                            piece_recip(d, 3)
                            piece_mul(d, 2)
                            piece_mul(d, 3)
                        if len(pending) == 3:
                            emit_av(pending.pop(0))
                        if qc > 0:
                            if g == 2:
                                piece_recip(d, 0)
                                piece_recip(d, 1)
                                piece_mul(d, 0)
                                piece_mul(d, 1)
                            elif g in (4, 5, 6, 7):
                                piece_out(d, g - 4, nc.sync)
                                if g == 7:
                                    avs.pop(d)
                                    for t in range(2):
                                        nc.sync.dma_start(
                                            out_r3[:, t, bass.ts(d, QC)],
                                            out_sb[:, t, bass.ts(d, QC)])
                        # exps
                        nc.scalar.activation(exa[:], sa[:], EXP)
                        nc.vector.tensor_scalar(exb[:], sb[:], SCHR_A,
                                                SCHR_B, op0=MULT, op1=ADD)
                        # low 16 bits of each fp32 = bf16 pattern of e^x
                        exbl = exb[:].bitcast(i16).rearrange(
                            "p (q two) -> p two q", two=2)
                        nc.vector.tensor_copy(exc[:], exbl[:, 0, :])
                        pending.append((exa, exc, qc, g))
                # drain last 3 av stages + chunk 3 epilogue (transposes
                # split across the Sync and ScalarE DMA queues)
                for _ in range(3):
                    emit_av(pending.pop(0))
                d = NCHUNK - 1
                for qb in range(4):
                    piece_recip(d, qb)
                for qb in range(4):
                    piece_mul(d, qb)
                for qb in range(4):
                    piece_out(d, qb, nc.sync if qb % 2 == 0 else nc.scalar)
                avs.pop(d)
                for t in range(2):
                    nc.sync.dma_start(out_r3[:, t, bass.ts(d, QC)],
                                      out_sb[:, t, bass.ts(d, QC)])

    nc.compile()
    return nc


def _get_built(skip_bv):
    if skip_bv not in _BUILT:
        _BUILT[skip_bv] = _build(skip_bv)
    return _BUILT[skip_bv]


def kernel(topview, sideview, Wq, bq, Wk, bk, Wv, bv):
    from concourse.bass_utils import run_bass_kernel_spmd

    topview = np.asarray(topview, dtype=np.float32)
    sideview = np.asarray(sideview, dtype=np.float32)
    wqT = np.ascontiguousarray(np.asarray(Wq, np.float32).T)
    wkT = np.ascontiguousarray(np.asarray(Wk, np.float32).T)
    wvT = np.ascontiguousarray(np.asarray(Wv, np.float32).T)
    bq = np.asarray(bq, np.float32).reshape(C8, 1)
    bk = np.asarray(bk, np.float32).reshape(C8, 1)
    bv = np.asarray(bv, np.float32).reshape(C, 1)

    top_f = topview.reshape(B, C, N)
    side_f = sideview.reshape(B, C, N)

    in_maps = []
    for core in range(NCORES):
        b, h = core // 2, core % 2
        in_maps.append({
            "top": np.ascontiguousarray(top_f[b, :, h * NQ:(h + 1) * NQ]),
            "side": np.ascontiguousarray(side_f[b]),
            "wqT": wqT, "wkT": wkT, "wvT": wvT,
            "bq": bq, "bk": bk, "bv": bv,
        })

    global _last_in_maps
    _last_in_maps = in_maps

    nc = _get_built(not np.any(np.asarray(bv)))
    res = run_bass_kernel_spmd(nc, in_maps, core_ids=list(range(NCORES)))

    out = np.empty((B, C, N), dtype=np.float32)
    for core in range(NCORES):
        b, h = core // 2, core % 2
        out[b, :, h * NQ:(h + 1) * NQ] = res.results[core]["out"]
    return out.reshape(B, C, H, W)
